# revision 1
# baseline (speedup 1.0000x reference)
"""Diagonal-Gaussian KL loss on 8 Trainium2 NeuronCores.

KL(p || q) summed over batch, with diag covariances exp(sigma):
  0.5 * [ sum(sigma_q - sigma_p) + sum(exp(sigma_p - sigma_q))
          + sum((mu_q-mu_p)^2 * exp(-sigma_q)) - B*D ]

Data-parallel over the batch dim: each core reduces a [1024, 2048] shard of
the four inputs to three per-partition partial sums; the tiny final combine
(8 cores x 128 partitions x 3 terms) happens on the host in float64.

The four inputs are stacked host-side into one [4, ROWS, D] tensor so each
[128, 2048] row-tile arrives in a single 4MB DMA.

Raw-bass pipeline (explicit semaphores; Tile was not usable here because
this walrus build allows only ONE sem-wait per compute/DMA instruction and
Tile's scheduler routinely emits two):
  per row-tile i (8 per core), with a 3-slot DMA ring and 2-slot compute
  buffers:
    SYNC: big[i%3] <- DMA row-tile i            (waits: slot free)
    DVE : a = sigma_p - sigma_q
          d = mu_q - mu_p                        (+inc: big slot released)
          u = d * e3                             (waits: e3 ready)
    ACT : e3 = exp(-0.5*sigma_q)                 (+inc)
          id(a)   accum-> acc_a   (in-place, result discarded)
          exp(a)  accum-> acc_e   (in-place, result discarded)
          u^2     accum-> acc_m   (in-place)     (+inc)
  tail: DVE reduces acc_* [128,8] -> res [128,3], SYNC DMAs res out.
The kernel is HBM-bound (~32MB/core, ~90us at ~360GB/s); DVE (~55us) and
ACT (~65us) hide under the DMA stream.
"""

from contextlib import ExitStack

import numpy as np

import concourse.bass as bass
from concourse import mybir
from concourse.bass_utils import run_bass_kernel_spmd

B, D = 8192, 2048
NCORES = 8
ROWS = B // NCORES  # rows per core
P = 128  # SBUF partitions
NT = ROWS // P  # row-tiles per core

F32 = mybir.dt.float32


def _build_nc():
    nc = bass.Bass(trn_type="TRN2", target_bir_lowering=False)

    x = nc.dram_tensor("x", [4, ROWS, D], F32, kind="ExternalInput")
    out = nc.dram_tensor("out", [P, 3], F32, kind="ExternalOutput")

    Exp = mybir.ActivationFunctionType.Exp
    Square = mybir.ActivationFunctionType.Square
    Identity = mybir.ActivationFunctionType.Identity
    Alu = mybir.AluOpType
    X = mybir.AxisListType.X

    ctx = ExitStack()
    with ctx:
        big = [ctx.enter_context(nc.sbuf_tensor(f"big{k}", [P, 4 * D], F32)) for k in range(3)]
        a_b = [ctx.enter_context(nc.sbuf_tensor(f"a{j}", [P, D], F32)) for j in range(2)]
        d_b = [ctx.enter_context(nc.sbuf_tensor(f"d{j}", [P, D], F32)) for j in range(2)]
        u_b = [ctx.enter_context(nc.sbuf_tensor(f"u{j}", [P, D], F32)) for j in range(2)]
        e3_b = [ctx.enter_context(nc.sbuf_tensor(f"e3{j}", [P, D], F32)) for j in range(2)]
        acc_a = ctx.enter_context(nc.sbuf_tensor("acc_a", [P, NT], F32))
        acc_e = ctx.enter_context(nc.sbuf_tensor("acc_e", [P, NT], F32))
        acc_m = ctx.enter_context(nc.sbuf_tensor("acc_m", [P, NT], F32))
        res = ctx.enter_context(nc.sbuf_tensor("res", [P, 3], F32))

        ds = [ctx.enter_context(nc.semaphore(f"ds{k}")) for k in range(3)]
        v_sem = ctx.enter_context(nc.semaphore("v_sem"))
        a_sem = ctx.enter_context(nc.semaphore("a_sem"))
        g_sem = ctx.enter_context(nc.semaphore("g_sem"))
        out_sem = ctx.enter_context(nc.semaphore("out_sem"))

        # DRAM AP for row-tile i: partitions = rows r..r+127, free = (t, d).
        def x_tile_ap(i):
            return bass.AP(x, i * P * D, [[D, P], [ROWS * D, 4], [1, D]])

        with nc.Block() as block:

            @block.sync
            def _(sync):
                for i in range(NT):
                    k = i % 3
                    if i >= 3:
                        # big[k]'s previous tile released by all three readers
                        sync.wait_ge(v_sem, 2 * (i - 3) + 1)
                        sync.wait_ge(a_sem, 2 * (i - 3) + 1)
                        sync.wait_ge(g_sem, (i - 3) + 1)
                    sync.dma_start(big[k][:, :], x_tile_ap(i)).then_inc(ds[k], 16)
                sync.wait_ge(v_sem, 2 * NT + 1)  # res written
                sync.dma_start(out[:, :], res[:, :]).then_inc(out_sem, 16)
                sync.wait_ge(out_sem, 16)

            @block.vector
            def _(vector):
                for i in range(NT):
                    k, j = i % 3, i % 2
                    vector.wait_ge(ds[k], 16 * (i // 3 + 1))  # tile i arrived
                    if i >= 2:
                        # a[j] freed by A2(i-2), u[j] freed by A3(i-2)
                        vector.wait_ge(a_sem, 2 * (i - 2) + 2)
                    sq_t = big[k][:, 0:D]
                    sp_t = big[k][:, D : 2 * D]
                    vector.tensor_sub(a_b[j][:, :], sp_t, sq_t)
                    vector.tensor_reduce(
                        acc_a[:, i : i + 1], a_b[j][:, :], axis=X, op=Alu.add
                    ).then_inc(v_sem, 1)
                    vector.wait_ge(g_sem, i + 1)  # d(i) ready
                    vector.wait_ge(a_sem, 2 * i + 1)  # e3(i) ready
                    vector.tensor_mul(
                        u_b[j][:, :], d_b[j][:, :], e3_b[j][:, :]
                    ).then_inc(v_sem, 1)
                vector.wait_ge(a_sem, 2 * NT)  # all accums final
                vector.tensor_reduce(res[:, 0:1], acc_a[:, :], axis=X, op=Alu.add)
                vector.tensor_reduce(res[:, 1:2], acc_e[:, :], axis=X, op=Alu.add)
                vector.tensor_reduce(res[:, 2:3], acc_m[:, :], axis=X, op=Alu.add).then_inc(v_sem, 1)

            @block.gpsimd
            def _(gpsimd):
                for i in range(NT):
                    k, j = i % 3, i % 2
                    gpsimd.wait_ge(ds[k], 16 * (i // 3 + 1))  # tile i arrived
                    if i >= 2:
                        gpsimd.wait_ge(v_sem, 2 * (i - 2) + 2)  # d[j] freed by V3
                    mq_t = big[k][:, 2 * D : 3 * D]
                    mp_t = big[k][:, 3 * D : 4 * D]
                    gpsimd.tensor_sub(d_b[j][:, :], mq_t, mp_t).then_inc(g_sem, 1)

            @block.scalar
            def _(scalar):
                for i in range(NT):
                    k, j = i % 3, i % 2
                    scalar.wait_ge(ds[k], 16 * (i // 3 + 1))  # sigma_q(i) arrived
                    if i >= 2:
                        scalar.wait_ge(v_sem, 2 * (i - 2) + 2)  # e3[j] freed
                    scalar.activation(
                        e3_b[j][:, :], big[k][:, 0:D], Exp, scale=-0.5
                    ).then_inc(a_sem, 1)
                    scalar.wait_ge(v_sem, 2 * i + 1)  # a(i) ready (V1+Ra done)
                    scalar.activation(
                        a_b[j][:, :], a_b[j][:, :], Exp,
                        accum_out=acc_e[:, i : i + 1],
                    )
                    scalar.wait_ge(v_sem, 2 * i + 2)  # u(i) ready
                    scalar.activation(
                        u_b[j][:, :], u_b[j][:, :], Square,
                        accum_out=acc_m[:, i : i + 1],
                    ).then_inc(a_sem, 1)

    return nc


_NC = None


def _get_nc():
    global _NC
    if _NC is None:
        _NC = _build_nc()
    return _NC


def _run(inputs, **kw):
    full = np.stack(
        [
            np.asarray(inputs["sigma_q"], dtype=np.float32),
            np.asarray(inputs["sigma_p"], dtype=np.float32),
            np.asarray(inputs["mu_q"], dtype=np.float32),
            np.asarray(inputs["mu_p"], dtype=np.float32),
        ],
        axis=0,
    )  # [4, B, D]
    in_maps = [
        {"x": np.ascontiguousarray(full[:, c * ROWS : (c + 1) * ROWS, :])}
        for c in range(NCORES)
    ]
    return run_bass_kernel_spmd(_get_nc(), in_maps, core_ids=list(range(NCORES)), **kw)


def _combine(results):
    # [8, 128, 3] partial sums -> scalar, in f64 for a clean final reduction
    S = np.stack([r["out"] for r in results]).astype(np.float64)
    s_a = S[..., 0].sum()
    s_e = S[..., 1].sum()
    s_m = S[..., 2].sum()
    kl = 0.5 * (-s_a + s_e + s_m - B * D)
    return np.asarray(kl, dtype=np.float32)


def kernel(**inputs):
    return _combine(_run(inputs).results)


def run_traced(inputs, **kw):
    """test.py helper: returns (value, BassKernelResults) with profiling."""
    br = _run(inputs, trace=True, **kw)
    return _combine(br.results), br



# revision 4
# speedup vs baseline: 1.2688x; 1.2688x over previous
"""Diagonal-Gaussian KL loss on 8 Trainium2 NeuronCores.

KL(p || q) summed over batch, with diag covariances exp(sigma):
  0.5 * [ sum(sigma_q - sigma_p) + sum(exp(sigma_p - sigma_q))
          + sum((mu_q-mu_p)^2 * exp(-sigma_q)) - B*D ]

Data-parallel over the batch dim: each core reduces a [1024, 2048] shard of
the four inputs to per-partition partial sums; the tiny final combine
happens on the host in float64.

The kernel is HBM-bound, so inputs are downcast host-side to bfloat16 and
stacked into one [4, ROWS, D] tensor; each [128, 4*2048] row-tile arrives in
a single 2MB DMA.  The 2e-2 rel-err budget dwarfs the ~0.4% bf16 noise
(which largely cancels across 33M elements anyway).

Raw-bass pipeline (explicit semaphores; this walrus build allows only ONE
sem-wait per compute/DMA instruction, waits are standalone seq ops):
  per row-tile i (8 per core), 3-slot DMA ring, 2-slot compute buffers:
    SYNC: big[i%3] <- DMA row-tile i
    DVE : stt  a = sp - sq, acc_a[i] = sum(a)     (fused sub+reduce)
          u = d * e3
          stt  acc_m[i] = sum(u*u)                (fused mul+reduce)
    ACT : e3 = exp(-0.5*sigma_q)
          exp(a) in-place, acc_t[i] = sum(exp(a)) (accum_out)
    POOL: d = mu_q - mu_p
  tail: SYNC DMAs the [128, 24] acc block out.
Only Exp is used on ACT (single activation table, loaded once).
"""

from contextlib import ExitStack

import ml_dtypes
import numpy as np

import concourse.bass as bass
from concourse import mybir
from concourse.bass_utils import run_bass_kernel_spmd

B, D = 8192, 2048
NCORES = 8
ROWS = B // NCORES  # rows per core
P = 128  # SBUF partitions
NT = ROWS // P  # row-tiles per core

F32 = mybir.dt.float32
BF16 = mybir.dt.bfloat16

A0, T0, M0 = 0, NT, 2 * NT  # acc column groups


def _build_nc():
    nc = bass.Bass(trn_type="TRN2", target_bir_lowering=False)

    x = nc.dram_tensor("x", [4, ROWS, D], BF16, kind="ExternalInput")
    out = nc.dram_tensor("out", [P, 3 * NT], F32, kind="ExternalOutput")

    Exp = mybir.ActivationFunctionType.Exp
    Alu = mybir.AluOpType

    ctx = ExitStack()
    with ctx:
        big = [ctx.enter_context(nc.sbuf_tensor(f"big{k}", [P, 4 * D], BF16)) for k in range(3)]
        a_b = [ctx.enter_context(nc.sbuf_tensor(f"a{j}", [P, D], BF16)) for j in range(2)]
        d_b = [ctx.enter_context(nc.sbuf_tensor(f"d{j}", [P, D], BF16)) for j in range(2)]
        e3_b = [ctx.enter_context(nc.sbuf_tensor(f"e3{j}", [P, D], BF16)) for j in range(2)]
        u_b = ctx.enter_context(nc.sbuf_tensor("u", [P, D], BF16))
        scrap = ctx.enter_context(nc.sbuf_tensor("scrap", [P, D], BF16))
        acc = ctx.enter_context(nc.sbuf_tensor("acc", [P, 3 * NT], F32))

        ds = [ctx.enter_context(nc.semaphore(f"ds{k}")) for k in range(3)]
        v_sem = ctx.enter_context(nc.semaphore("v_sem"))  # DVE: stt_a, ttr per tile
        a_sem = ctx.enter_context(nc.semaphore("a_sem"))  # ACT: e3, exp_acc per tile
        g_sem = ctx.enter_context(nc.semaphore("g_sem"))  # POOL: d per tile
        out_sem = ctx.enter_context(nc.semaphore("out_sem"))

        # DRAM AP for row-tile i: partitions = rows r..r+127, free = (t, d).
        def x_tile_ap(i):
            return bass.AP(x, i * P * D, [[D, P], [ROWS * D, 4], [1, D]])

        with nc.Block() as block:

            @block.sync
            def _(sync):
                for i in range(NT):
                    k = i % 3
                    if i >= 3:
                        # big[k]'s previous tile released by all three readers
                        sync.wait_ge(v_sem, 2 * (i - 3) + 1)
                        sync.wait_ge(a_sem, 2 * (i - 3) + 1)
                        sync.wait_ge(g_sem, (i - 3) + 1)
                    sync.dma_start(big[k][:, :], x_tile_ap(i)).then_inc(ds[k], 16)
                sync.wait_ge(v_sem, 2 * NT)  # ttr(NT-1): acc_a, acc_m final
                sync.wait_ge(a_sem, 2 * NT)  # exp_acc(NT-1): acc_t final
                sync.dma_start(out[:, :], acc[:, :]).then_inc(out_sem, 16)
                sync.wait_ge(out_sem, 16)

            @block.vector
            def _(vector):
                for i in range(NT):
                    k, j = i % 3, i % 2
                    vector.wait_ge(ds[k], 16 * (i // 3 + 1))  # tile i arrived
                    if i >= 2:
                        vector.wait_ge(a_sem, 2 * (i - 2) + 2)  # a[j] freed by exp_acc(i-2)
                    sq_t = big[k][:, 0:D]
                    sp_t = big[k][:, D : 2 * D]
                    vector.scalar_tensor_tensor(
                        a_b[j][:, :], sp_t, 0.0, sq_t, Alu.bypass, Alu.subtract,
                        accum_out=acc[:, A0 + i : A0 + i + 1],
                    ).then_inc(v_sem, 1)
                    vector.wait_ge(a_sem, 2 * i + 1)  # e3(i) ready
                    vector.wait_ge(g_sem, i + 1)  # d(i) ready
                    vector.tensor_mul(u_b[:, :], d_b[j][:, :], e3_b[j][:, :])
                    vector.scalar_tensor_tensor(
                        scrap[:, :], u_b[:, :], 0.0, u_b[:, :],
                        Alu.bypass, Alu.mult,
                        accum_out=acc[:, M0 + i : M0 + i + 1],
                    ).then_inc(v_sem, 1)

            @block.scalar
            def _(scalar):
                for i in range(NT):
                    k, j = i % 3, i % 2
                    scalar.wait_ge(ds[k], 16 * (i // 3 + 1))  # sigma_q(i) arrived
                    if i >= 2:
                        scalar.wait_ge(v_sem, 2 * (i - 2) + 2)  # e3[j] freed by u(i-2)
                    scalar.activation(
                        e3_b[j][:, :], big[k][:, 0:D], Exp, scale=-0.5
                    ).then_inc(a_sem, 1)
                    scalar.wait_ge(v_sem, 2 * i + 1)  # a(i) ready
                    scalar.activation(
                        a_b[j][:, :], a_b[j][:, :], Exp,
                        accum_out=acc[:, T0 + i : T0 + i + 1],
                    ).then_inc(a_sem, 1)

            @block.gpsimd
            def _(gpsimd):
                for i in range(NT):
                    k, j = i % 3, i % 2
                    gpsimd.wait_ge(ds[k], 16 * (i // 3 + 1))  # tile i arrived
                    if i >= 2:
                        gpsimd.wait_ge(v_sem, 2 * (i - 2) + 2)  # d[j] freed by u(i-2)
                    mq_t = big[k][:, 2 * D : 3 * D]
                    mp_t = big[k][:, 3 * D : 4 * D]
                    gpsimd.tensor_sub(d_b[j][:, :], mq_t, mp_t).then_inc(g_sem, 1)

    return nc


_NC = None


def _get_nc():
    global _NC
    if _NC is None:
        _NC = _build_nc()
    return _NC


def _run(inputs, **kw):
    full = np.stack(
        [
            np.asarray(inputs["sigma_q"], dtype=np.float32).astype(ml_dtypes.bfloat16),
            np.asarray(inputs["sigma_p"], dtype=np.float32).astype(ml_dtypes.bfloat16),
            np.asarray(inputs["mu_q"], dtype=np.float32).astype(ml_dtypes.bfloat16),
            np.asarray(inputs["mu_p"], dtype=np.float32).astype(ml_dtypes.bfloat16),
        ],
        axis=0,
    )  # [4, B, D] bf16
    in_maps = [
        {"x": np.ascontiguousarray(full[:, c * ROWS : (c + 1) * ROWS, :])}
        for c in range(NCORES)
    ]
    return run_bass_kernel_spmd(_get_nc(), in_maps, core_ids=list(range(NCORES)), **kw)


def _combine(results):
    # [8, 128, 3*NT] partial sums -> scalar, in f64 for a clean final reduction
    S = np.stack([r["out"] for r in results]).astype(np.float64)
    s_a = S[..., A0:T0].sum()  # sum(sigma_p - sigma_q)
    s_t = S[..., T0:M0].sum()  # sum(exp(sigma_p - sigma_q))
    s_m = S[..., M0:].sum()  # sum((mu_q-mu_p)^2 exp(-sigma_q))
    kl = 0.5 * (-s_a + s_t + s_m - B * D)
    return np.asarray(kl, dtype=np.float32)


def kernel(**inputs):
    return _combine(_run(inputs).results)


def run_traced(inputs, **kw):
    """test.py helper: returns (value, BassKernelResults) with profiling."""
    br = _run(inputs, trace=True, **kw)
    return _combine(br.results), br


# revision 5
# speedup vs baseline: 1.3355x; 1.0526x over previous
"""Diagonal-Gaussian KL loss on 8 Trainium2 NeuronCores.

KL(p || q) summed over batch, with diag covariances exp(sigma):
  0.5 * [ sum(sigma_q - sigma_p) + sum(exp(sigma_p - sigma_q))
          + sum((mu_q-mu_p)^2 * exp(-sigma_q)) - B*D ]

Data-parallel over the batch dim: each core reduces a [1024, 2048] shard of
the four inputs to per-partition partial sums; the tiny final combine
happens on the host in float64.

The kernel is HBM-bound, so inputs are downcast host-side to bfloat16 and
stacked into one [4, ROWS, D] tensor; each [128, 4*2048] row-tile arrives in
a single 2MB DMA.  The 2e-2 rel-err budget dwarfs the ~0.4% bf16 noise
(which largely cancels across 33M elements anyway).

Measured HW op costs ([128,2048] tiles): DVE stt 2.5us, DVE tensor_tensor
4.1us (half rate - avoid), ACT activation ~2.0us, Pool sub 5.8us, DMA
6.3us/tile.  Assignment keeps every engine under the DMA stream rate:
  per row-tile i (8 per core), 3-slot DMA ring, 2-slot compute buffers:
    SYNC: big[i%3] <- DMA row-tile i
    DVE : stt  a = sp - sq, acc_a[i] = sum(a)       (fused sub+reduce)
          stt  u = d * e3
    ACT : e3 = exp(-0.5*sigma_q)
          exp(a) in-place, acc_t[i] = sum(exp(a))   (accum_out)
          square(u) in-place, acc_m[i] = sum(u^2)   (accum_out)
    POOL: d = mu_q - mu_p
  tail: SYNC DMAs the [128, 24] acc block out.
Exp and Square share one activation table (exp_and_others) - loaded once.
"""

from contextlib import ExitStack

import ml_dtypes
import numpy as np

import concourse.bass as bass
from concourse import mybir
from concourse.bass_utils import run_bass_kernel_spmd

B, D = 8192, 2048
NCORES = 8
ROWS = B // NCORES  # rows per core
P = 128  # SBUF partitions
NT = ROWS // P  # row-tiles per core

F32 = mybir.dt.float32
BF16 = mybir.dt.bfloat16

A0, T0, M0 = 0, NT, 2 * NT  # acc column groups


def _build_nc():
    nc = bass.Bass(trn_type="TRN2", target_bir_lowering=False)

    x = nc.dram_tensor("x", [4, ROWS, D], BF16, kind="ExternalInput")
    out = nc.dram_tensor("out", [P, 3 * NT], F32, kind="ExternalOutput")

    Exp = mybir.ActivationFunctionType.Exp
    Square = mybir.ActivationFunctionType.Square
    Alu = mybir.AluOpType

    ctx = ExitStack()
    with ctx:
        big = [ctx.enter_context(nc.sbuf_tensor(f"big{k}", [P, 4 * D], BF16)) for k in range(3)]
        a_b = [ctx.enter_context(nc.sbuf_tensor(f"a{j}", [P, D], BF16)) for j in range(2)]
        d_b = [ctx.enter_context(nc.sbuf_tensor(f"d{j}", [P, D], BF16)) for j in range(2)]
        e3_b = [ctx.enter_context(nc.sbuf_tensor(f"e3{j}", [P, D], BF16)) for j in range(2)]
        u_b = [ctx.enter_context(nc.sbuf_tensor(f"u{j}", [P, D], BF16)) for j in range(2)]
        acc = ctx.enter_context(nc.sbuf_tensor("acc", [P, 3 * NT], F32))

        ds = [ctx.enter_context(nc.semaphore(f"ds{k}")) for k in range(3)]
        v_sem = ctx.enter_context(nc.semaphore("v_sem"))  # DVE: stt_a, stt_u per tile
        a_sem = ctx.enter_context(nc.semaphore("a_sem"))  # ACT: e3, exp_acc, sq_acc
        g_sem = ctx.enter_context(nc.semaphore("g_sem"))  # POOL: d per tile
        out_sem = ctx.enter_context(nc.semaphore("out_sem"))

        # DRAM AP for row-tile i: partitions = rows r..r+127, free = (t, d).
        def x_tile_ap(i):
            return bass.AP(x, i * P * D, [[D, P], [ROWS * D, 4], [1, D]])

        with nc.Block() as block:

            @block.sync
            def _(sync):
                for i in range(NT):
                    k = i % 3
                    if i >= 3:
                        # big[k]'s previous tile released by all three readers
                        sync.wait_ge(v_sem, 2 * (i - 3) + 1)  # stt_a(i-3) read sq,sp
                        sync.wait_ge(a_sem, 3 * (i - 3) + 1)  # e3(i-3) read sq
                        sync.wait_ge(g_sem, (i - 3) + 1)  # d(i-3) read mq,mp
                    sync.dma_start(big[k][:, :], x_tile_ap(i)).then_inc(ds[k], 16)
                sync.wait_ge(v_sem, 2 * NT - 1)  # stt_a(NT-1): acc_a final
                sync.wait_ge(a_sem, 3 * NT)  # sq_acc(NT-1): acc_t, acc_m final
                sync.dma_start(out[:, :], acc[:, :]).then_inc(out_sem, 16)
                sync.wait_ge(out_sem, 16)

            @block.vector
            def _(vector):
                for i in range(NT):
                    k, j = i % 3, i % 2
                    vector.wait_ge(ds[k], 16 * (i // 3 + 1))  # tile i arrived
                    if i >= 2:
                        vector.wait_ge(a_sem, 3 * (i - 2) + 2)  # a[j] freed by exp_acc(i-2)
                    sq_t = big[k][:, 0:D]
                    sp_t = big[k][:, D : 2 * D]
                    vector.scalar_tensor_tensor(
                        a_b[j][:, :], sp_t, 0.0, sq_t, Alu.bypass, Alu.subtract,
                        accum_out=acc[:, A0 + i : A0 + i + 1],
                    ).then_inc(v_sem, 1)
                    vector.wait_ge(a_sem, 3 * i + 1)  # e3(i) ready (also: u[j] free)
                    vector.wait_ge(g_sem, i + 1)  # d(i) ready
                    vector.scalar_tensor_tensor(
                        u_b[j][:, :], d_b[j][:, :], 0.0, e3_b[j][:, :],
                        Alu.bypass, Alu.mult,
                    ).then_inc(v_sem, 1)

            @block.scalar
            def _(scalar):
                for i in range(NT):
                    k, j = i % 3, i % 2
                    scalar.wait_ge(ds[k], 16 * (i // 3 + 1))  # sigma_q(i) arrived
                    if i >= 2:
                        scalar.wait_ge(v_sem, 2 * (i - 2) + 2)  # e3[j] freed by stt_u(i-2)
                    scalar.activation(
                        e3_b[j][:, :], big[k][:, 0:D], Exp, scale=-0.5
                    ).then_inc(a_sem, 1)
                    scalar.wait_ge(v_sem, 2 * i + 1)  # a(i) ready
                    scalar.activation(
                        a_b[j][:, :], a_b[j][:, :], Exp,
                        accum_out=acc[:, T0 + i : T0 + i + 1],
                    ).then_inc(a_sem, 1)
                    scalar.wait_ge(v_sem, 2 * i + 2)  # u(i) ready
                    scalar.activation(
                        u_b[j][:, :], u_b[j][:, :], Square,
                        accum_out=acc[:, M0 + i : M0 + i + 1],
                    ).then_inc(a_sem, 1)

            @block.gpsimd
            def _(gpsimd):
                for i in range(NT):
                    k, j = i % 3, i % 2
                    gpsimd.wait_ge(ds[k], 16 * (i // 3 + 1))  # tile i arrived
                    if i >= 2:
                        gpsimd.wait_ge(v_sem, 2 * (i - 2) + 2)  # d[j] freed by stt_u(i-2)
                    mq_t = big[k][:, 2 * D : 3 * D]
                    mp_t = big[k][:, 3 * D : 4 * D]
                    gpsimd.tensor_sub(d_b[j][:, :], mq_t, mp_t).then_inc(g_sem, 1)

    return nc


_NC = None


def _get_nc():
    global _NC
    if _NC is None:
        _NC = _build_nc()
    return _NC


def _run(inputs, **kw):
    full = np.stack(
        [
            np.asarray(inputs["sigma_q"], dtype=np.float32).astype(ml_dtypes.bfloat16),
            np.asarray(inputs["sigma_p"], dtype=np.float32).astype(ml_dtypes.bfloat16),
            np.asarray(inputs["mu_q"], dtype=np.float32).astype(ml_dtypes.bfloat16),
            np.asarray(inputs["mu_p"], dtype=np.float32).astype(ml_dtypes.bfloat16),
        ],
        axis=0,
    )  # [4, B, D] bf16
    in_maps = [
        {"x": np.ascontiguousarray(full[:, c * ROWS : (c + 1) * ROWS, :])}
        for c in range(NCORES)
    ]
    return run_bass_kernel_spmd(_get_nc(), in_maps, core_ids=list(range(NCORES)), **kw)


def _combine(results):
    # [8, 128, 3*NT] partial sums -> scalar, in f64 for a clean final reduction
    S = np.stack([r["out"] for r in results]).astype(np.float64)
    s_a = S[..., A0:T0].sum()  # sum(sigma_p - sigma_q)
    s_t = S[..., T0:M0].sum()  # sum(exp(sigma_p - sigma_q))
    s_m = S[..., M0:].sum()  # sum((mu_q-mu_p)^2 exp(-sigma_q))
    kl = 0.5 * (-s_a + s_t + s_m - B * D)
    return np.asarray(kl, dtype=np.float32)


def kernel(**inputs):
    return _combine(_run(inputs).results)


def run_traced(inputs, **kw):
    """test.py helper: returns (value, BassKernelResults) with profiling."""
    br = _run(inputs, trace=True, **kw)
    return _combine(br.results), br


# revision 6
# speedup vs baseline: 1.6023x; 1.1997x over previous
"""Diagonal-Gaussian KL loss on 8 Trainium2 NeuronCores.

KL(p || q) summed over batch, with diag covariances exp(sigma):
  0.5 * [ sum(sigma_q - sigma_p) + sum(exp(sigma_p - sigma_q))
          + sum((mu_q-mu_p)^2 * exp(-sigma_q)) - B*D ]

Data-parallel over the batch dim: each core reduces a [1024, 2048] shard of
the four inputs to per-partition partial sums; the tiny final combine
happens on the host in float64.

The kernel is HBM-bound, so inputs are downcast host-side to bfloat16 and
stacked into one [4, ROWS, D] tensor; each [128, 4*2048] row-tile arrives in
a single 2MB DMA.  The 2e-2 rel-err budget dwarfs the ~0.4% bf16 noise.

Measured HW op costs ([128,2048] tiles): DVE stt ~2.3-2.5us, ACT activation
~2.0us, Pool sub 5.8us (too slow - moved to PE), DMA 6.3us/tile.
  per row-tile i (8 per core), 3-slot DMA ring, 2-slot compute buffers:
    SYNC: big[i%3] <- DMA row-tile i
    PE  : d_ps[j] = I @ mu_q_tile + (-I) @ mu_p_tile   (PSUM f32, 4 chunks)
    DVE : stt  a = sp - sq, acc_a[i] = sum(a)          (fused sub+reduce)
          stt  u = d_ps * e3
    ACT : e3 = exp(-0.5*sigma_q)
          exp(a) in-place, acc_t[i] = sum(exp(a))      (accum_out)
          square(u) in-place, acc_m[i] = sum(u^2)      (accum_out)
  tail: SYNC DMAs the [128, 24] acc block out.
Exp and Square share one activation table (exp_and_others) - loaded once.
"""

from contextlib import ExitStack

import ml_dtypes
import numpy as np

import concourse.bass as bass
from concourse import mybir
from concourse.bass_utils import run_bass_kernel_spmd

B, D = 8192, 2048
NCORES = 8
ROWS = B // NCORES  # rows per core
P = 128  # SBUF partitions
NT = ROWS // P  # row-tiles per core
NC = D // 512  # 512-col PSUM chunks per tile

F32 = mybir.dt.float32
BF16 = mybir.dt.bfloat16

A0, T0, M0 = 0, NT, 2 * NT  # acc column groups


def _build_nc():
    nc = bass.Bass(trn_type="TRN2", target_bir_lowering=False)

    x = nc.dram_tensor("x", [4, ROWS, D], BF16, kind="ExternalInput")
    cst = nc.dram_tensor("cst", [P, 2 * P], BF16, kind="ExternalInput")  # [I | -I]
    out = nc.dram_tensor("out", [P, 3 * NT], F32, kind="ExternalOutput")

    Exp = mybir.ActivationFunctionType.Exp
    Square = mybir.ActivationFunctionType.Square
    Alu = mybir.AluOpType

    ctx = ExitStack()
    with ctx:
        big = [ctx.enter_context(nc.sbuf_tensor(f"big{k}", [P, 4 * D], BF16)) for k in range(3)]
        a_b = [ctx.enter_context(nc.sbuf_tensor(f"a{j}", [P, D], BF16)) for j in range(2)]
        e3_b = [ctx.enter_context(nc.sbuf_tensor(f"e3{j}", [P, D], BF16)) for j in range(2)]
        u_b = [ctx.enter_context(nc.sbuf_tensor(f"u{j}", [P, D], BF16)) for j in range(2)]
        w_b = ctx.enter_context(nc.sbuf_tensor("w", [P, 2 * P], BF16))
        acc = ctx.enter_context(nc.sbuf_tensor("acc", [P, 3 * NT], F32))
        d_ps = [ctx.enter_context(nc.psum_tensor(f"dps{j}", [P, D], F32)) for j in range(2)]

        ds = [ctx.enter_context(nc.semaphore(f"ds{k}")) for k in range(3)]
        cs = ctx.enter_context(nc.semaphore("cs"))
        v_sem = ctx.enter_context(nc.semaphore("v_sem"))  # DVE: stt_a, stt_u per tile
        a_sem = ctx.enter_context(nc.semaphore("a_sem"))  # ACT: e3, exp_acc, sq_acc
        pe_sem = ctx.enter_context(nc.semaphore("pe_sem"))  # PE: d per tile
        out_sem = ctx.enter_context(nc.semaphore("out_sem"))

        # DRAM AP for row-tile i: partitions = rows r..r+127, free = (t, d).
        def x_tile_ap(i):
            return bass.AP(x, i * P * D, [[D, P], [ROWS * D, 4], [1, D]])

        with nc.Block() as block:

            @block.sync
            def _(sync):
                sync.dma_start(w_b[:, :], cst[:, :]).then_inc(cs, 16)
                for i in range(NT):
                    k = i % 3
                    if i >= 3:
                        # big[k]'s previous tile released by all three readers
                        sync.wait_ge(v_sem, 2 * (i - 3) + 1)  # stt_a(i-3) read sq,sp
                        sync.wait_ge(a_sem, 3 * (i - 3) + 1)  # e3(i-3) read sq
                        sync.wait_ge(pe_sem, (i - 3) + 1)  # PE d(i-3) read mq,mp
                    sync.dma_start(big[k][:, :], x_tile_ap(i)).then_inc(ds[k], 16)
                sync.wait_ge(v_sem, 2 * NT - 1)  # stt_a(NT-1): acc_a final
                sync.wait_ge(a_sem, 3 * NT)  # sq_acc(NT-1): acc_t, acc_m final
                sync.dma_start(out[:, :], acc[:, :]).then_inc(out_sem, 16)
                sync.wait_ge(out_sem, 16)

            @block.tensor
            def _(pe):
                pe.wait_ge(cs, 16)
                for i in range(NT):
                    k, j = i % 3, i % 2
                    pe.wait_ge(ds[k], 16 * (i // 3 + 1))  # mu_q/mu_p(i) arrived
                    if i >= 2:
                        pe.wait_ge(v_sem, 2 * (i - 2) + 2)  # d_ps[j] freed by stt_u(i-2)
                    for c in range(NC):
                        cols = slice(c * 512, (c + 1) * 512)
                        mm = pe.matmul(
                            d_ps[j][:, cols], w_b[:, 0:P],
                            big[k][:, 2 * D : 3 * D][:, cols],
                            start=True, stop=False,
                        )
                        mm = pe.matmul(
                            d_ps[j][:, cols], w_b[:, P : 2 * P],
                            big[k][:, 3 * D : 4 * D][:, cols],
                            start=False, stop=True,
                        )
                        if c == NC - 1:
                            mm.then_inc(pe_sem, 1)

            @block.vector
            def _(vector):
                for i in range(NT):
                    k, j = i % 3, i % 2
                    vector.wait_ge(ds[k], 16 * (i // 3 + 1))  # tile i arrived
                    if i >= 2:
                        vector.wait_ge(a_sem, 3 * (i - 2) + 2)  # a[j] freed by exp_acc(i-2)
                    sq_t = big[k][:, 0:D]
                    sp_t = big[k][:, D : 2 * D]
                    vector.scalar_tensor_tensor(
                        a_b[j][:, :], sp_t, 0.0, sq_t, Alu.bypass, Alu.subtract,
                        accum_out=acc[:, A0 + i : A0 + i + 1],
                    ).then_inc(v_sem, 1)
                    vector.wait_ge(a_sem, 3 * i + 1)  # e3(i) ready (also: u[j] free)
                    vector.wait_ge(pe_sem, i + 1)  # d_ps(i) ready
                    vector.scalar_tensor_tensor(
                        u_b[j][:, :], d_ps[j][:, :], 0.0, e3_b[j][:, :],
                        Alu.bypass, Alu.mult,
                    ).then_inc(v_sem, 1)

            @block.scalar
            def _(scalar):
                for i in range(NT):
                    k, j = i % 3, i % 2
                    scalar.wait_ge(ds[k], 16 * (i // 3 + 1))  # sigma_q(i) arrived
                    if i >= 2:
                        scalar.wait_ge(v_sem, 2 * (i - 2) + 2)  # e3[j] freed by stt_u(i-2)
                    scalar.activation(
                        e3_b[j][:, :], big[k][:, 0:D], Exp, scale=-0.5
                    ).then_inc(a_sem, 1)
                    scalar.wait_ge(v_sem, 2 * i + 1)  # a(i) ready
                    scalar.activation(
                        a_b[j][:, :], a_b[j][:, :], Exp,
                        accum_out=acc[:, T0 + i : T0 + i + 1],
                    ).then_inc(a_sem, 1)
                    scalar.wait_ge(v_sem, 2 * i + 2)  # u(i) ready
                    scalar.activation(
                        u_b[j][:, :], u_b[j][:, :], Square,
                        accum_out=acc[:, M0 + i : M0 + i + 1],
                    ).then_inc(a_sem, 1)

    return nc


_NC = None


def _get_nc():
    global _NC
    if _NC is None:
        _NC = _build_nc()
    return _NC


def _consts():
    eye = np.eye(P, dtype=np.float32)
    return np.concatenate([eye, -eye], axis=1).astype(ml_dtypes.bfloat16)


def _run(inputs, **kw):
    full = np.stack(
        [
            np.asarray(inputs["sigma_q"], dtype=np.float32).astype(ml_dtypes.bfloat16),
            np.asarray(inputs["sigma_p"], dtype=np.float32).astype(ml_dtypes.bfloat16),
            np.asarray(inputs["mu_q"], dtype=np.float32).astype(ml_dtypes.bfloat16),
            np.asarray(inputs["mu_p"], dtype=np.float32).astype(ml_dtypes.bfloat16),
        ],
        axis=0,
    )  # [4, B, D] bf16
    cst = _consts()
    in_maps = [
        {
            "x": np.ascontiguousarray(full[:, c * ROWS : (c + 1) * ROWS, :]),
            "cst": cst,
        }
        for c in range(NCORES)
    ]
    return run_bass_kernel_spmd(_get_nc(), in_maps, core_ids=list(range(NCORES)), **kw)


def _combine(results):
    # [8, 128, 3*NT] partial sums -> scalar, in f64 for a clean final reduction
    S = np.stack([r["out"] for r in results]).astype(np.float64)
    s_a = S[..., A0:T0].sum()  # sum(sigma_p - sigma_q)
    s_t = S[..., T0:M0].sum()  # sum(exp(sigma_p - sigma_q))
    s_m = S[..., M0:].sum()  # sum((mu_q-mu_p)^2 exp(-sigma_q))
    kl = 0.5 * (-s_a + s_t + s_m - B * D)
    return np.asarray(kl, dtype=np.float32)


def kernel(**inputs):
    return _combine(_run(inputs).results)


def run_traced(inputs, **kw):
    """test.py helper: returns (value, BassKernelResults) with profiling."""
    br = _run(inputs, trace=True, **kw)
    return _combine(br.results), br
